# revision 2
# baseline (speedup 1.0000x reference)
"""Trainium2 Bass kernel for nn_NDNRefinement (4-layer GNN message passing), v2.

Strategy (8 NeuronCores), building on the v1 baseline but eliminating nearly
all per-row indirect DMA:
- Triples sorted by s_idx, sharded by subject owner. Subject rows are
  expanded with one-hot matmuls from an SBUF-resident local table (static
  128-aligned windows, 1-3 source tiles per 512-block, host-verified).
- Object rows come from a deduplicated per-(src,dst) "slab" exchange: each
  core gathers the distinct rows its peers need (one dma_gather per peer),
  AllToAll's the slabs, and phase A fetches per-entry rows feature-major with
  one dma_gather(transpose=True) per 512-block. This replaces both the
  per-entry indirect gathers and the AllGather of the full table.
- new_o rows are scattered into o-sorted, quantile-normalized bucket slots
  (indirect DMA, 128 rows/instr) and exchanged with AllToAll as in v1.
- Pooling streams stage/recv buckets sequentially (contiguous DMA) and pools
  with one-hot matmuls against statically scheduled PSUM accumulators; the
  (chunk -> tile) schedule is the union across cores so the program is SPMD.
- Per-object MLPs run on 6-tile groups; transposes use the DVE 32x32 path.
"""

import os
import hashlib
import numpy as np
import ml_dtypes

import concourse.bass as bass
import concourse.bacc as bacc
import concourse.tile as tile
from concourse import mybir
from concourse.bass_utils import run_bass_kernel_spmd

BF16 = mybir.dt.bfloat16
F32 = mybir.dt.float32
I32 = mybir.dt.int32
I16 = mybir.dt.int16
P = 128
NC = 8
ALPHA = 0.2
SLAB = 4096          # rows per (src,dst) dedup slab; 8*SLAB == 32768 (int16)
GROUP = 4            # object tiles pooled concurrently (PSUM banks)

DIMS = [(64, 512, 128), (128, 512, 128), (128, 512, 128), (128, 128, 128)]


def _rup(x, m):
    return ((int(x) + m - 1) // m) * m


def _wrap16(idx, ncols):
    """index list -> [128, ncols] int16 (16-wrap, replicated to 128 parts)."""
    a = np.zeros((16, ncols), np.int16)
    for i, v in enumerate(idx):
        a[i % 16, i // 16] = v
    return np.tile(a, (8, 1))


# ---------------------------------------------------------------------------
# Host preprocessing
# ---------------------------------------------------------------------------

def preprocess(inputs):
    obj_vecs = np.asarray(inputs["obj_vecs"], np.float32)
    pred_vecs = np.asarray(inputs["pred_vecs"], np.float32)
    pred_boxes = np.asarray(inputs["pred_boxes"], np.float32)
    s_idx = np.asarray(inputs["s_idx"], np.int32)
    o_idx = np.asarray(inputs["o_idx"], np.int32)

    O = obj_vecs.shape[0]
    assert O % NC == 0
    OS = O // NC
    OSP = _rup(OS, P)
    NT = OSP // P
    bf = ml_dtypes.bfloat16

    order = np.argsort(s_idx, kind="stable")
    s_sorted = s_idx[order]
    o_sorted = o_idx[order]
    bnd = np.searchsorted(s_sorted, np.arange(NC + 1) * OS)
    counts_c = np.diff(bnd)
    T_PC = max(_rup(counts_c.max(), 512), 512)
    NB = T_PC // 512
    NCH = T_PC // P

    cnt = np.bincount(s_idx, minlength=O) + np.bincount(o_idx, minlength=O)
    inv_cnt = (1.0 / np.maximum(cnt, 1)).astype(np.float32)

    # ---- per-core raw data -------------------------------------------------
    cores = []
    for c in range(NC):
        sl = slice(bnd[c], bnd[c + 1])
        idxs = order[sl]
        n = len(idxs)
        s_loc = (s_sorted[sl] - c * OS).astype(np.int64)
        o_glob = o_sorted[sl].astype(np.int64)
        d_c = o_glob // OS
        o_loc = o_glob - d_c * OS
        # padded views
        s_pad = np.concatenate([s_loc, np.full(T_PC - n, s_loc[-1] if n else 0,
                                               np.int64)])
        cores.append(dict(idxs=idxs, n=n, s_loc=s_loc, o_loc=o_loc, d_c=d_c,
                          s_pad=s_pad))

    # ---- s-expand windows (union across cores, static) --------------------
    allpad = np.stack([pc["s_pad"] for pc in cores]).reshape(NC, NB, 512)
    bmin = allpad.min(2).min(0)
    bmax = allpad.max(2).max(0)
    wtile = (bmin // P).astype(np.int64)
    nw = (bmax // P - wtile + 1).astype(np.int64)
    assert nw.max() <= 3, f"s window too wide: {nw.max()}"

    # ---- o dedup request lists + entry idx16 -------------------------------
    # req[c][d] = sorted distinct o_loc that core c needs from core d
    req = [[None] * NC for _ in range(NC)]
    for c in range(NC):
        pc = cores[c]
        for d in range(NC):
            u = np.unique(pc["o_loc"][pc["d_c"] == d])
            assert len(u) <= SLAB, f"slab overflow {len(u)}"
            req[c][d] = u

    # ---- new_o bucket slots (quantile-normalized, o-sorted) ----------------
    S_B_need = 0
    for c in range(NC):
        bc_ = np.bincount(cores[c]["d_c"], minlength=NC)
        S_B_need = max(S_B_need, int(bc_.max()) if cores[c]["n"] else 0)
    S_B = _rup(S_B_need + P, P)

    slot_of = []          # per core: array [n] -> d*S_B + slot
    slots_dc = {}         # (c_src, d_dst) -> (slots array, o_loc array)
    for c in range(NC):
        pc = cores[c]
        n = pc["n"]
        slot_arr = np.zeros(n, np.int64)
        used = np.zeros((NC, S_B), bool)
        for d in range(NC):
            m = np.nonzero(pc["d_c"] == d)[0]
            ol = pc["o_loc"][m]
            osort = np.argsort(ol, kind="stable")
            mi = m[osort]
            ol_s = ol[osort]
            k = len(mi)
            slots = np.zeros(k, np.int64)
            prev = -1
            for i in range(k):
                s = max(prev + 1, int(ol_s[i]) * S_B // OSP)
                slots[i] = s
                prev = s
            # clamp from top so slots fit
            for i in range(k - 1, -1, -1):
                cap = S_B - (k - i)
                if slots[i] > cap:
                    slots[i] = cap
                else:
                    break
            assert k == 0 or (np.diff(slots) > 0).all()
            assert k == 0 or (slots[0] >= 0 and slots[-1] < S_B)
            slot_arr[mi] = d * S_B + slots
            used[d, slots] = True
            slots_dc[(c, d)] = (slots, ol_s)
        slot_of.append((slot_arr, used))

    # ---- phase C schedules (union across cores) ----------------------------
    # s-side: chunk ch covers entries [128ch,128ch+128)
    s_tiles = []       # per chunk: sorted tile list
    for ch in range(NCH):
        ts = set()
        for c in range(NC):
            pc = cores[c]
            a, b = ch * P, min((ch + 1) * P, pc["n"])
            if a < b:
                ts |= set((pc["s_loc"][a:b] // P).tolist())
        s_tiles.append(sorted(ts))

    # o-side: bucket d chunk k covers slots [128k, 128k+128)
    NKB = S_B // P
    o_tiles = {}       # (d, k) -> sorted tile list
    for d in range(NC):
        for k in range(NKB):
            ts = set()
            for c in range(NC):
                slots, ol_s = slots_dc[(d, c)]  # rows d sends to c? careful!
                ts = ts
            o_tiles[(d, k)] = ts
    # NOTE: receiving core c's bucket d holds rows produced by core d destined
    # to c: slots_dc[(d, c)]. Union over receiving cores is what we need, but
    # the schedule is identical for all cores (SPMD): for bucket-slot d, chunk
    # k, tiles = union over receivers c of tiles touched.
    for d in range(NC):
        for k in range(NKB):
            ts = set()
            for c in range(NC):
                slots, ol_s = slots_dc[(d, c)]
                m = (slots >= k * P) & (slots < (k + 1) * P)
                if m.any():
                    ts |= set((ol_s[m] // P).tolist())
            o_tiles[(d, k)] = sorted(ts)

    # group structure
    n_groups = -(-NT // GROUP)
    groups = []
    ncols = 0
    col_entries = []   # list of per-column info to build ploc param per core
    for g in range(n_groups):
        tiles = list(range(g * GROUP, min((g + 1) * GROUP, NT)))
        tset = set(tiles)
        # s chunks touching this group
        chs = [ch for ch in range(NCH) if set(s_tiles[ch]) & tset]
        s_items = {}
        if chs:
            c0, c1 = min(chs), max(chs) + 1
        else:
            c0, c1 = 0, 0
        for ch in range(c0, c1):
            items = []
            for t in s_tiles[ch]:
                if t in tset:
                    items.append((t, ncols))
                    col_entries.append(("s", ch, t))
                    ncols += 1
            s_items[ch] = items
        # batches of <= 8 chunks
        s_batches = [(b, min(b + 8, c1)) for b in range(c0, c1, 8)]
        # o side
        o_ranges = {}
        o_items = {}
        for d in range(NC):
            ks = [k for k in range(NKB) if set(o_tiles[(d, k)]) & tset]
            if ks:
                k0, k1 = min(ks), max(ks) + 1
            else:
                k0, k1 = 0, 0
            # split into batches of <=8 chunks
            o_ranges[d] = [(b, min(b + 8, k1)) for b in range(k0, k1, 8)]
            for k in range(k0, k1):
                items = []
                for t in o_tiles[(d, k)]:
                    if t in tset:
                        items.append((t, ncols))
                        col_entries.append(("o", d, k, t))
                        ncols += 1
                o_items[(d, k)] = items
        groups.append(dict(tiles=tuple(tiles), c0=c0, c1=c1,
                           s_batches=tuple(s_batches),
                           s_items=tuple((ch, tuple(s_items[ch]))
                                         for ch in range(c0, c1)),
                           o_ranges=tuple((d, tuple(o_ranges[d]))
                                          for d in range(NC)),
                           o_items=tuple(((d, k), tuple(o_items[(d, k)]))
                                         for d in range(NC)
                                         for k in range(NKB)
                                         if (d, k) in o_items)))
    NCOLS = _rup(max(ncols, 1), 4)

    # matmul counts per (group, tile) for start/stop flags, per sub-phase
    acc_s, acc_o = {}, {}
    for g, gi in enumerate(groups):
        for ch, items in gi["s_items"]:
            for (t, _) in items:
                acc_s[(g, t)] = acc_s.get((g, t), 0) + 1
        for (dk, items) in gi["o_items"]:
            for (t, _) in items:
                acc_o[(g, t)] = acc_o.get((g, t), 0) + 1
    for g, gi in enumerate(groups):
        for t in gi["tiles"]:
            assert acc_s.get((g, t), 0) > 0, f"empty s tile {g},{t}"
    acc_n = acc_s

    cfg = dict(O=O, OS=OS, OSP=OSP, NT=NT, T_PC=T_PC, NB=NB, NCH=NCH,
               S_B=S_B, NKB=NKB, NCOLS=NCOLS, SCW=-(-NB // 3) * 512,
               wtile=tuple(int(x) for x in wtile),
               nw=tuple(int(x) for x in nw),
               groups=tuple(
                   tuple(sorted(gi.items())) for gi in groups),
               acc_s=tuple(sorted(acc_s.items())),
               acc_o=tuple(sorted(acc_o.items())))

    # ---- shared weights ----------------------------------------------------
    shared = {}
    shared["w_emb"] = np.asarray(inputs["W_emb"], np.float32).astype(bf)
    shared["b_emb"] = np.asarray(inputs["b_emb"], np.float32).reshape(-1, 1)
    for li, (din, h, dout) in enumerate(DIMS):
        b1b = np.asarray(inputs[f"b1b{li}"], np.float32)
        shared[f"w1a{li}"] = np.asarray(inputs[f"W1a{li}"], np.float32).astype(bf)
        shared[f"w1b{li}"] = np.asarray(inputs[f"W1b{li}"], np.float32).astype(bf)
        shared[f"w2a{li}"] = np.asarray(inputs[f"W2a{li}"], np.float32).astype(bf)
        shared[f"w2b{li}"] = np.asarray(inputs[f"W2b{li}"], np.float32).astype(bf)
        shared[f"b1a{li}"] = np.asarray(inputs[f"b1a{li}"], np.float32).reshape(-1, P).T.copy()
        shared[f"b1bp{li}"] = b1b[h:h + dout].reshape(-1, 1).copy()
        shared[f"b1bs{li}"] = np.broadcast_to(b1b[:h].astype(bf), (P, h)).copy()
        shared[f"b1bo{li}"] = np.broadcast_to(b1b[h + dout:].astype(bf), (P, h)).copy()
        shared[f"b2a{li}"] = np.asarray(inputs[f"b2a{li}"], np.float32).reshape(-1, P).T.copy()
        shared[f"b2b{li}"] = np.asarray(inputs[f"b2b{li}"], np.float32).reshape(-1, 1).copy()
    shared["wbb"] = np.asarray(inputs["W_bb"], np.float32).astype(bf)
    shared["bbb"] = np.asarray(inputs["b_bb"], np.float32).reshape(-1, 1)
    # constants
    iotac = np.zeros((P, 4), np.float32)
    iotac[:, 0] = np.arange(P) - 128
    iotac[:, 1] = np.arange(P)
    iotac[:, 2] = np.arange(P) + 128
    shared["iotac"] = iotac.astype(bf)
    shared["iotai"] = np.broadcast_to(np.arange(P, dtype=np.int32), (P, P)).copy()
    shared["identbf"] = np.eye(P, dtype=np.float32).astype(bf)

    # ---- per-core parameter maps -------------------------------------------
    x_full = np.concatenate([obj_vecs, pred_boxes], axis=1)
    NFC = -(-(NC * S_B - T_PC) // P)  # worst-case free-slot columns
    cfg["NFC"] = NFC

    in_maps = []
    for c in range(NC):
        pc = cores[c]
        n = pc["n"]
        m = {}
        xT = np.zeros((68, OSP), bf)
        xT[:, :OS] = x_full[c * OS:(c + 1) * OS].T.astype(bf)
        m["xt"] = xT
        pT = np.zeros((64, T_PC), bf)
        pT[:, :n] = pred_vecs[pc["idxs"]].T.astype(bf)
        m["pred0"] = pT
        # sloc param [3, SCW] bf16, centered so integers stay exact
        SCW = -(-NB // 3) * 512
        sloc = np.zeros((3, SCW), np.float32)
        for j in range(NB):
            v = pc["s_pad"][j * 512:(j + 1) * 512] - wtile[j] * P
            assert (v >= 0).all() and (v < 384).all()
            sloc[j % 3, (j // 3) * 512:(j // 3) * 512 + 512] = v - 128
        m["sloc"] = sloc.astype(bf)
        # oi16: per entry idx into recv slab
        idx16 = np.zeros(T_PC, np.int64)
        for d in range(NC):
            mm_ = np.nonzero(pc["d_c"] == d)[0]
            pos = np.searchsorted(req[c][d], pc["o_loc"][mm_])
            idx16[mm_] = d * SLAB + (pos % P) * (SLAB // P) + pos // P
        assert idx16.max() < 32768
        oi = np.zeros((16, NB * 32), np.int16)
        for j in range(NB):
            seg = idx16[j * 512:(j + 1) * 512]
            for i, v in enumerate(seg):
                oi[i % 16, 32 * j + i // 16] = v
        m["oi16"] = np.tile(oi, (8, 1))
        # gl16: what each peer cc needs from me (c as owner)
        gl = np.zeros((16, NC * (SLAB // 16)), np.int16)
        for cc in range(NC):
            u = req[cc][c]
            for i, v in enumerate(u):
                gl[i % 16, cc * (SLAB // 16) + i // 16] = v
        m["gl16"] = np.tile(gl, (8, 1))
        # obi scatter targets [128, NB*4] i32
        slot_arr, used = slot_of[c]
        ob_ = np.zeros(T_PC, np.int32)
        ob_[:n] = slot_arr.astype(np.int32)
        npad = T_PC - n
        if npad:
            free = np.nonzero(~used.reshape(-1))[0]
            assert len(free) >= npad
            ob_[n:] = free[:npad].astype(np.int32)
        obcols = np.zeros((NB * 4, P), np.int32)
        for j in range(NB):
            seg = ob_[j * 512:(j + 1) * 512]
            obcols[4 * j:4 * j + 4] = seg.reshape(4, P)
        m["obi"] = obcols.T.copy()
        # remaining unwritten slots, zero-filled by the kernel
        rest = free[npad:]
        fz = np.zeros((NFC * P,), np.int32)
        fz[:len(rest)] = rest
        # pad duplicates replaced by first pad slot? no: pad with rows beyond
        # data is fine only if distinct; reuse 'rest' then fill tail with
        # slots already zeroed this call (harmless WW avoided by staying
        # distinct): tail gets remaining frees cyclically offset -> ensure
        # distinct by construction (NFC*P <= len(free)-npad asserted)
        assert NFC * P <= max(len(rest), 1) or len(rest) == NFC * P or True
        if len(rest) < NFC * P:
            fz[len(rest):] = rest[:NFC * P - len(rest)] if len(rest) else 0
        m["fzero"] = fz.reshape(NFC, P).T.copy()
        # ploc pooling one-hot columns [128, NCOLS] i32
        ploc = np.full((NCOLS, P), -1, np.int32)
        # s-side positions: entry e of chunk ch at partition e%128
        for colidx, ent in enumerate(col_entries):
            if ent[0] == "s":
                _, ch, t = ent
                a = ch * P
                col = np.full(P, -1, np.int64)
                b = min(a + P, n)
                if a < b:
                    v = pc["s_loc"][a:b]
                    in_t = (v // P) == t
                    col[:b - a][in_t] = v[in_t] - t * P
                ploc[colidx] = col
            else:
                _, d, k, t = ent
                slots, ol_s = slots_dc[(d, c)]
                col = np.full(P, -1, np.int64)
                mm_ = (slots >= k * P) & (slots < (k + 1) * P)
                sl_ = slots[mm_] - k * P
                ov_ = ol_s[mm_]
                in_t = (ov_ // P) == t
                col[sl_[in_t]] = ov_[in_t] - t * P
                ploc[colidx] = col
        m["ploc"] = ploc.T.copy()
        iv = np.zeros((OSP,), np.float32)
        iv[:OS] = inv_cnt[c * OS:(c + 1) * OS]
        m["invc"] = iv.reshape(NT, P).T.copy()
        m.update(shared)
        in_maps.append(m)

    return cfg, in_maps


# ---------------------------------------------------------------------------
# Kernel builder
# ---------------------------------------------------------------------------

def build_kernel(cfg):
    OSP, NT, T_PC = cfg["OSP"], cfg["NT"], cfg["T_PC"]
    NB, NCH, S_B, NKB = cfg["NB"], cfg["NCH"], cfg["S_B"], cfg["NKB"]
    NCOLS = cfg["NCOLS"]
    wtile, nw = cfg["wtile"], cfg["nw"]
    groups = [dict(g) for g in cfg["groups"]]
    acc_s = dict(cfg["acc_s"])
    acc_o = dict(cfg["acc_o"])

    nc = bacc.Bacc("TRN2", target_bir_lowering=False, debug=False,
                   num_devices=NC)

    # ---- parameters ----
    xt = nc.declare_dram_parameter("xt", [68, OSP], BF16, isOutput=False)
    pred0 = nc.declare_dram_parameter("pred0", [64, T_PC], BF16, isOutput=False)
    SCW = cfg["SCW"]
    sloc_p = nc.declare_dram_parameter("sloc", [3, SCW], BF16, isOutput=False)
    oi16_p = nc.declare_dram_parameter("oi16", [P, NB * 32], I16, isOutput=False)
    gl16_p = nc.declare_dram_parameter("gl16", [P, NC * (SLAB // 16)], I16,
                                       isOutput=False)
    obi_p = nc.declare_dram_parameter("obi", [P, NB * 4], I32, isOutput=False)
    ploc_p = nc.declare_dram_parameter("ploc", [P, NCOLS], I32, isOutput=False)
    invc_p = nc.declare_dram_parameter("invc", [P, NT], F32, isOutput=False)
    NFC = cfg["NFC"]
    fzero_p = nc.declare_dram_parameter("fzero", [P, NFC], I32, isOutput=False)
    iotac_p = nc.declare_dram_parameter("iotac", [P, 4], BF16, isOutput=False)
    iotai_p = nc.declare_dram_parameter("iotai", [P, P], I32, isOutput=False)
    ident_p = nc.declare_dram_parameter("identbf", [P, P], BF16, isOutput=False)

    w_emb = nc.declare_dram_parameter("w_emb", [68, 64], BF16, isOutput=False)
    b_emb = nc.declare_dram_parameter("b_emb", [64, 1], F32, isOutput=False)
    wp = {}
    for li, (din, h, dout) in enumerate(DIMS):
        wp[f"w1a{li}"] = nc.declare_dram_parameter(f"w1a{li}", [3 * din, h], BF16, isOutput=False)
        wp[f"w1b{li}"] = nc.declare_dram_parameter(f"w1b{li}", [h, 2 * h + dout], BF16, isOutput=False)
        wp[f"w2a{li}"] = nc.declare_dram_parameter(f"w2a{li}", [h, h], BF16, isOutput=False)
        wp[f"w2b{li}"] = nc.declare_dram_parameter(f"w2b{li}", [h, dout], BF16, isOutput=False)
        wp[f"b1a{li}"] = nc.declare_dram_parameter(f"b1a{li}", [P, h // P], F32, isOutput=False)
        wp[f"b1bp{li}"] = nc.declare_dram_parameter(f"b1bp{li}", [dout, 1], F32, isOutput=False)
        wp[f"b1bs{li}"] = nc.declare_dram_parameter(f"b1bs{li}", [P, h], BF16, isOutput=False)
        wp[f"b1bo{li}"] = nc.declare_dram_parameter(f"b1bo{li}", [P, h], BF16, isOutput=False)
        wp[f"b2a{li}"] = nc.declare_dram_parameter(f"b2a{li}", [P, h // P], F32, isOutput=False)
        wp[f"b2b{li}"] = nc.declare_dram_parameter(f"b2b{li}", [dout, 1], F32, isOutput=False)
    wbb = nc.declare_dram_parameter("wbb", [P, 4], BF16, isOutput=False)
    bbb = nc.declare_dram_parameter("bbb", [4, 1], F32, isOutput=False)

    out = nc.declare_dram_parameter("out", [4, OSP], F32, isOutput=True)

    # ---- internal DRAM ----
    agins = [nc.dram_tensor(f"agins{li}", [OSP, P], BF16) for li in range(4)]
    sslab = [nc.dram_tensor(f"sslab{li}", [NC, P, SLAB // P * P], BF16)
             for li in range(4)]
    rslab = [nc.dram_tensor(f"rslab{li}", [NC, P, SLAB // P * P], BF16)
             for li in range(4)]
    stages, sends, recvs = [], [], []
    for li, (din, h, dout) in enumerate(DIMS):
        stages.append(nc.dram_tensor(f"stage{li}", [T_PC, h], BF16))
        sends.append(nc.dram_tensor(f"send{li}", [NC * S_B, h], BF16))
        recvs.append(nc.dram_tensor(f"recv{li}", [NC * S_B, h], BF16))
    preds = [pred0]
    for li in range(1, 4):
        preds.append(nc.dram_tensor(f"pred{li}", [P, T_PC], BF16))
    partd = [nc.dram_tensor(f"partd{li}", [P, NT * DIMS[li][1]], BF16)
             for li in range(4)]

    PRELU = mybir.ActivationFunctionType.Prelu
    COPY = mybir.ActivationFunctionType.Copy
    ISEQ = mybir.AluOpType.is_equal
    GRPS = [list(range(NC))]

    _MAXL = int(os.environ.get("KMAXL", "4"))
    _NOPOOL = bool(int(os.environ.get("KNOPOOL", "0")))
    _NOMLP = bool(int(os.environ.get("KNOMLP", "0")))
    _NOSLAB = bool(int(os.environ.get("KNOSLAB", "0")))
    _NOSGATH = bool(int(os.environ.get("KNOSGATH", "0")))

    with tile.TileContext(nc) as tc:
        with tc.tile_pool(name="cst", bufs=1) as cst:
            W = {}

            def load_w(name, src_ap, hh, ww, dt):
                t = cst.tile([hh, ww], dt, tag=name, name=name)
                nc.sync.dma_start(out=t[:], in_=src_ap)
                W[name] = t

            load_w("w_emb", w_emb[:, :], 68, 64, BF16)
            load_w("b_emb", b_emb[:, :], 64, 1, F32)
            load_w("wbb", wbb[:, :], P, 4, BF16)
            load_w("bbb", bbb[:, :], 4, 1, F32)
            load_w("invc", invc_p[:, :], P, NT, F32)
            load_w("iotac", iotac_p[:, :], P, 4, BF16)
            load_w("iotai", iotai_p[:, :], P, P, I32)
            load_w("ident", ident_p[:, :], P, P, BF16)
            sloc_sb = cst.tile([65, SCW], BF16, tag="sloc_sb", name="sloc_sb")
            ones_sb = cst.tile([65, P], BF16, tag="ones_sb", name="ones_sb")
            for rr in range(3):
                nc.sync.dma_start(out=sloc_sb[32 * rr:32 * rr + 1, :],
                                  in_=sloc_p[rr:rr + 1, :])
                nc.vector.memset(ones_sb[32 * rr:32 * rr + 1, :], 1.0)
            load_w("oi16", oi16_p[:, :], P, NB * 32, I16)
            load_w("gl16", gl16_p[:, :], P, NC * (SLAB // 16), I16)
            load_w("obi", obi_p[:, :], P, NB * 4, I32)
            load_w("ploc", ploc_p[:, :], P, NCOLS, I32)
            load_w("fzero", fzero_p[:, :], P, NFC, I32)
            for li, (din, h, dout) in enumerate(DIMS):
                for ki in range(3):
                    load_w(f"w1a{li}_c{ki}", wp[f"w1a{li}"][ki * din:(ki + 1) * din, :],
                           din, h, BF16)
                for k in range(h // P):
                    load_w(f"w1b{li}_{k}", wp[f"w1b{li}"][k * P:(k + 1) * P, :],
                           P, 2 * h + dout, BF16)
                    load_w(f"w2a{li}_{k}", wp[f"w2a{li}"][k * P:(k + 1) * P, :],
                           P, h, BF16)
                    load_w(f"w2b{li}_{k}", wp[f"w2b{li}"][k * P:(k + 1) * P, :],
                           P, dout, BF16)
                load_w(f"b1a{li}", wp[f"b1a{li}"][:, :], P, h // P, F32)
                load_w(f"b1bp{li}", wp[f"b1bp{li}"][:, :], dout, 1, F32)
                load_w(f"b1bs{li}", wp[f"b1bs{li}"][:, :], P, h, BF16)
                load_w(f"b1bo{li}", wp[f"b1bo{li}"][:, :], P, h, BF16)
                load_w(f"b2a{li}", wp[f"b2a{li}"][:, :], P, h // P, F32)
                load_w(f"b2b{li}", wp[f"b2b{li}"][:, :], dout, 1, F32)

            # persistent local tables (row-major, tile-major columns)
            tabRM_a = cst.tile([P, NT * 64], BF16, tag="tabRM_a", name="tabRM_a")
            tabRM_b = cst.tile([P, NT * P], BF16, tag="tabRM_b", name="tabRM_b")

            # ---------------- embedding ----------------
            NEB = -(-OSP // 512)
            with (
                tc.tile_pool(name="esb", bufs=3) as esb,
                tc.tile_pool(name="eps", bufs=3, space="PSUM") as eps,
            ):
                zero64 = esb.tile([P, 64], BF16, tag="zero64", name="zero64")
                nc.vector.memset(zero64[:], 0.0)
                for b in range(NEB):
                    c0 = b * 512
                    w = min(512, OSP - c0)
                    xin = esb.tile([68, 512], BF16, tag="xin")
                    nc.sync.dma_start(out=xin[:, :w], in_=xt[:, c0:c0 + w])
                    pse = eps.tile([64, 512], F32, space="PSUM", tag="pse")
                    nc.tensor.matmul(out=pse[:, :w], lhsT=W["w_emb"][:],
                                     rhs=xin[:, :w], start=True, stop=True)
                    ebt = esb.tile([64, 512], BF16, tag="ebt")
                    nc.scalar.activation(out=ebt[:, :w], in_=pse[:, :w],
                                         func=PRELU, bias=W["b_emb"][:, :1],
                                         alpha=ALPHA)
                    for q in range(-(-w // P)):
                        qw = min(P, w - q * P)
                        t_glob = 4 * b + q
                        ptr = eps.tile([P, 64], BF16, space="PSUM", tag="ptr")
                        nc.tensor.transpose(out=ptr[:qw, :],
                                            in_=ebt[:, q * P:q * P + qw],
                                            identity=W["ident"][:64, :64])
                        ent = esb.tile([P, P], BF16, tag="ent")
                        nc.vector.tensor_copy(out=ent[:qw, :64], in_=ptr[:qw, :])
                        nc.vector.tensor_copy(out=ent[:qw, 64:], in_=zero64[:qw, :])
                        nc.vector.tensor_copy(
                            out=tabRM_a[:qw, t_glob * 64:(t_glob + 1) * 64],
                            in_=ptr[:qw, :])
                        nc.sync.dma_start(
                            out=agins[0][c0 + q * P:c0 + q * P + qw, :],
                            in_=ent[:qw, :])

            # ---------------- layers ----------------
            for li, (din, h, dout) in enumerate(DIMS[:_MAXL]):
                NH = h // P
                pred_in = preds[li]
                stage, send, recv = stages[li], sends[li], recvs[li]
                s_cols = (0, h)
                p_cols = (h, h + dout)
                o_cols = (h + dout, 2 * h + dout)

                # ---- slab exchange for this layer's o-side inputs ----
                if not _NOSLAB:
                    with tc.tile_pool(name=f"gsb{li}", bufs=2) as gsb:
                        if not _NOSGATH:
                            for cc in range(NC):
                                gout = gsb.tile([P, SLAB // P, P], BF16, tag="gout")
                                nc.gpsimd.dma_gather(
                                    gout[:], agins[li][:],
                                    W["gl16"][:, cc * (SLAB // 16):(cc + 1) * (SLAB // 16)],
                                    SLAB, SLAB, P, transpose=False,
                                    single_packet=False)
                                nc.sync.dma_start(
                                    out=sslab[li][cc],
                                    in_=gout[:].rearrange("p a b -> p (a b)"))
                    nc.gpsimd.collective_compute(
                        "AllToAll", mybir.AluOpType.bypass, replica_groups=GRPS,
                        ins=[sslab[li][:]], outs=[rslab[li][:]])

                if _NOMLP:
                    continue

                # ---- phase A: triple MLP ----
                with (
                    tc.tile_pool(name=f"asb{li}", bufs=3) as asb,
                    tc.tile_pool(name=f"apsm{li}", bufs=1, space="PSUM") as aps_m,
                    tc.tile_pool(name=f"apsh{li}", bufs=NH, space="PSUM") as aps_hid,
                    tc.tile_pool(name=f"apso{li}", bufs=2, space="PSUM") as aps_out,
                ):
                    zrow = asb.tile([P, h], BF16, tag="zrow", name="zrow")
                    nc.vector.memset(zrow[:], 0.0)
                    for fc in range(NFC):
                        nc.gpsimd.indirect_dma_start(
                            out=send[:],
                            out_offset=bass.IndirectOffsetOnAxis(
                                ap=W["fzero"][:, fc:fc + 1], axis=0),
                            in_=zrow[:], in_offset=None)
                    for j in range(NB):
                        wt, nwj = wtile[j], nw[j]
                        bp = 32 * (j % 3)
                        bc = aps_m.tile([P, 512], F32, space="PSUM", tag="bc")
                        nc.tensor.matmul(
                            out=bc[:], lhsT=ones_sb[bp:bp + 1, :],
                            rhs=sloc_sb[bp:bp + 1,
                                        (j // 3) * 512:(j // 3) * 512 + 512],
                            start=True, stop=True)
                        ohs = []
                        for k in range(nwj):
                            oh = asb.tile([P, 512], BF16, tag=f"oh{k}",
                                          name=f"oh{k}")
                            nc.vector.tensor_tensor(
                                out=oh[:], in0=bc[:],
                                in1=W["iotac"][:, k:k + 1].to_broadcast([P, 512]),
                                op=ISEQ)
                            ohs.append(oh)
                        stp = aps_m.tile([P, 512], F32, space="PSUM", tag="stp")
                        for k in range(nwj):
                            nc.tensor.matmul(
                                out=stp[:din, :],
                                lhsT=tabRM_a[:, (wt + k) * 64:(wt + k + 1) * 64]
                                if li == 0 else
                                tabRM_b[:, (wt + k) * P:(wt + k + 1) * P],
                                rhs=ohs[k][:],
                                start=(k == 0), stop=(k == nwj - 1))
                        sT = asb.tile([P, 512], BF16, tag="sT")
                        nc.scalar.activation(out=sT[:din, :], in_=stp[:din, :],
                                             func=COPY)
                        oT = asb.tile([P, 1, 512], BF16, tag="oT")
                        nc.gpsimd.dma_gather(
                            oT[:],
                            rslab[li][:].rearrange("a p (k b) -> (a p k) b", b=P),
                            W["oi16"][:, 32 * j:32 * j + 32],
                            512, 512, P, transpose=True,
                            single_packet=False)
                        pT = asb.tile([P, 512], BF16, tag="pT")
                        nc.sync.dma_start(out=pT[:din, :],
                                          in_=pred_in[:din, 512 * j:512 * (j + 1)])
                        obi = asb.tile([P, 4], I32, tag="obi")
                        nc.vector.tensor_copy(out=obi[:],
                                              in_=W["obi"][:, 4 * j:4 * j + 4])

                        hidT = []
                        for mh in range(NH):
                            ph = aps_hid.tile([P, 512], F32, space="PSUM", tag="ph")
                            nc.tensor.matmul(
                                out=ph[:],
                                lhsT=W[f"w1a{li}_c0"][:, mh * P:(mh + 1) * P],
                                rhs=sT[:din, :], start=True, stop=False)
                            nc.tensor.matmul(
                                out=ph[:],
                                lhsT=W[f"w1a{li}_c1"][:, mh * P:(mh + 1) * P],
                                rhs=pT[:din, :], start=False, stop=False)
                            nc.tensor.matmul(
                                out=ph[:],
                                lhsT=W[f"w1a{li}_c2"][:, mh * P:(mh + 1) * P],
                                rhs=oT[:din, 0, :], start=False, stop=True)
                            ht = asb.tile([P, 512], BF16, tag=f"hidT{mh}",
                                          name=f"hidT{mh}")
                            nc.scalar.activation(out=ht[:], in_=ph[:], func=PRELU,
                                                 bias=W[f"b1a{li}"][:, mh:mh + 1],
                                                 alpha=ALPHA)
                            hidT.append(ht)

                        for (cols, bname, is_s) in ((s_cols, f"b1bs{li}", True),
                                                    (o_cols, f"b1bo{li}", False)):
                            for e in range(4):
                                po = aps_out.tile([P, 512], F32, space="PSUM",
                                                  tag="po")
                                for k in range(NH):
                                    nc.tensor.matmul(
                                        out=po[:, :h],
                                        lhsT=hidT[k][:, e * P:(e + 1) * P],
                                        rhs=W[f"w1b{li}_{k}"][:, cols[0]:cols[1]],
                                        start=(k == 0), stop=(k == NH - 1))
                                nc.vector.tensor_tensor(
                                    out=po[:, :h], in0=po[:, :h],
                                    in1=W[bname][:, :h], op=mybir.AluOpType.add)
                                ov = asb.tile([P, 512], BF16, tag="ov")
                                nc.scalar.activation(out=ov[:, :h], in_=po[:, :h],
                                                     func=PRELU, alpha=ALPHA)
                                if is_s:
                                    r0 = 512 * j + e * P
                                    nc.sync.dma_start(out=stage[r0:r0 + P, :],
                                                      in_=ov[:, :h])
                                else:
                                    nc.gpsimd.indirect_dma_start(
                                        out=send[:],
                                        out_offset=bass.IndirectOffsetOnAxis(
                                            ap=obi[:, e:e + 1], axis=0),
                                        in_=ov[:, :h], in_offset=None)

                        if li < 3:
                            pp = aps_out.tile([P, 512], F32, space="PSUM",
                                              tag="po")
                            for k in range(NH):
                                nc.tensor.matmul(
                                    out=pp[:dout, :],
                                    lhsT=W[f"w1b{li}_{k}"][:, p_cols[0]:p_cols[1]],
                                    rhs=hidT[k][:],
                                    start=(k == 0), stop=(k == NH - 1))
                            pv = asb.tile([dout, 512], BF16, tag="pv")
                            nc.scalar.activation(out=pv[:], in_=pp[:dout, :],
                                                 func=PRELU,
                                                 bias=W[f"b1bp{li}"][:, :1],
                                                 alpha=ALPHA)
                            nc.sync.dma_start(
                                out=preds[li + 1][:, 512 * j:512 * (j + 1)],
                                in_=pv[:])

                # ---- phase C-s: subject-side pooling into SBUF partials,
                # emitted before the A2A so it overlaps the collective ----
                with (
                    tc.tile_pool(name=f"css{li}", bufs=2) as css,
                    tc.tile_pool(name=f"cps{li}", bufs=GROUP, space="PSUM") as cps,
                ):
                    for g, gi in enumerate(groups):
                        tiles = gi["tiles"]
                        accs = {}
                        seen = {}
                        for t in tiles:
                            accs[t] = cps.tile([P, h], F32, space="PSUM",
                                               tag="acc", name=f"accs{t}")
                            seen[t] = 0
                        s_items = dict(gi["s_items"])
                        for (b0, b1) in gi["s_batches"]:
                            sv = css.tile([P, 8, h], BF16, tag="sv")
                            nc.sync.dma_start(
                                out=sv[:, :b1 - b0, :],
                                in_=stage[P * b0:P * b1, :]
                                .rearrange("(a p) d -> p a d", p=P))
                            for ch in range(b0, b1):
                                for (t, colidx) in s_items.get(ch, ()):
                                    first = seen[t] == 0
                                    seen[t] += 1
                                    last = seen[t] == acc_s[(g, t)]
                                    oh = css.tile([P, P], BF16, tag="ohp")
                                    nc.vector.tensor_tensor(
                                        out=oh[:],
                                        in0=W["ploc"][:, colidx:colidx + 1]
                                        .to_broadcast([P, P]),
                                        in1=W["iotai"][:], op=ISEQ)
                                    nc.tensor.matmul(
                                        out=accs[t][:], lhsT=oh[:],
                                        rhs=sv[:, ch - b0, :],
                                        start=first, stop=last)
                        pgrp = css.tile([P, GROUP, h], BF16, tag="pgrp")
                        for ti, t in enumerate(tiles):
                            nc.scalar.activation(
                                out=pgrp[:, ti, :],
                                in_=accs[t][:], func=COPY)
                        nc.sync.dma_start(
                            out=partd[li][:, tiles[0] * h:
                                          (tiles[0] + len(tiles)) * h],
                            in_=pgrp[:, :len(tiles), :]
                            .rearrange("p a d -> p (a d)"))

                    # ---- AllToAll of new_o rows (overlaps C-s above) ----
                    nc.gpsimd.collective_compute(
                        "AllToAll", mybir.AluOpType.bypass, replica_groups=GRPS,
                        ins=[send[:]], outs=[recv[:]])

                    if _NOPOOL:
                        continue

                    # ---- phase C-o: object-side pooling + object MLP ----
                    with (
                        tc.tile_pool(name=f"csb{li}", bufs=2) as csb,
                        tc.tile_pool(name=f"cmlp{li}", bufs=2, space="PSUM") as cmlp,
                        tc.tile_pool(name=f"ctr{li}", bufs=2, space="PSUM") as ctr,
                    ):
                        for g, gi in enumerate(groups):
                            tiles = gi["tiles"]
                            psl = csb.tile([P, GROUP * h], BF16, tag="psl")
                            nc.sync.dma_start(
                                out=psl[:, :len(tiles) * h],
                                in_=partd[li][:, tiles[0] * h:
                                              (tiles[0] + len(tiles)) * h])
                            accs = {}
                            seen = {}
                            for ti, t in enumerate(tiles):
                                accs[t] = cps.tile([P, h], F32, space="PSUM",
                                                   tag="acc", name=f"acco{t}")
                                nc.tensor.matmul(
                                    out=accs[t][:], lhsT=W["ident"][:],
                                    rhs=psl[:, ti * h:(ti + 1) * h],
                                    start=True,
                                    stop=acc_o.get((g, t), 0) == 0)
                                seen[t] = 0
                            o_items = dict(gi["o_items"])
                            for (d, rngs) in gi["o_ranges"]:
                                for (k0, k1) in rngs:
                                    ov2 = csb.tile([P, 8, h], BF16, tag="ovv")
                                    nc.sync.dma_start(
                                        out=ov2[:, :k1 - k0, :],
                                        in_=recv[d * S_B + P * k0:
                                                 d * S_B + P * k1, :]
                                        .rearrange("(a p) d -> p a d", p=P))
                                    for k in range(k0, k1):
                                        for (t, colidx) in o_items.get((d, k), ()):
                                            seen[t] += 1
                                            last = seen[t] == acc_o[(g, t)]
                                            oh = csb.tile([P, P], BF16,
                                                          tag="ohp2")
                                            nc.vector.tensor_tensor(
                                                out=oh[:],
                                                in0=W["ploc"][:, colidx:colidx + 1]
                                                .to_broadcast([P, P]),
                                                in1=W["iotai"][:], op=ISEQ)
                                            nc.tensor.matmul(
                                                out=accs[t][:], lhsT=oh[:],
                                                rhs=ov2[:, k - k0, :],
                                                start=False, stop=last)

                            # finish: scale, transpose, object MLP
                            pooledT = [csb.tile([P, GROUP * P], BF16,
                                                tag=f"plT{kk}", name=f"plT{kk}")
                                       for kk in range(NH)]
                            for ti, t in enumerate(tiles):
                                pob = csb.tile([P, h], BF16, tag="pob")
                                nc.scalar.activation(out=pob[:], in_=accs[t][:],
                                                     func=COPY,
                                                     scale=W["invc"][:, t:t + 1])
                                for kk in range(NH):
                                    ptr = ctr.tile([P, P], BF16, space="PSUM",
                                                   tag="ptr")
                                    nc.tensor.transpose(
                                        out=ptr[:],
                                        in_=pob[:, kk * P:(kk + 1) * P],
                                        identity=W["ident"][:])
                                    nc.vector.tensor_copy(
                                        out=pooledT[kk][:, ti * P:(ti + 1) * P],
                                        in_=ptr[:])

                            gw = len(tiles) * P
                            nbt = -(-gw // 384)
                            for bi in range(nbt):
                                o0 = bi * 384
                                bw = min(384, gw - o0)
                                hid2 = []
                                for mh in range(NH):
                                    p2 = cmlp.tile([P, 512], F32, space="PSUM",
                                                   tag="p2")
                                    for kk in range(NH):
                                        nc.tensor.matmul(
                                            out=p2[:, :bw],
                                            lhsT=W[f"w2a{li}_{kk}"][:, mh * P:(mh + 1) * P],
                                            rhs=pooledT[kk][:, o0:o0 + bw],
                                            start=(kk == 0), stop=(kk == NH - 1))
                                    h2 = csb.tile([P, 512], BF16, tag=f"h2_{mh}",
                                                  name=f"h2_{mh}")
                                    nc.scalar.activation(
                                        out=h2[:, :bw], in_=p2[:, :bw],
                                        func=PRELU,
                                        bias=W[f"b2a{li}"][:, mh:mh + 1],
                                        alpha=ALPHA)
                                    hid2.append(h2)
                                pno = cmlp.tile([P, 512], F32, space="PSUM",
                                                tag="p2", name="pno")
                                for kk in range(NH):
                                    nc.tensor.matmul(out=pno[:dout, :bw],
                                                     lhsT=W[f"w2b{li}_{kk}"][:],
                                                     rhs=hid2[kk][:, :bw],
                                                     start=(kk == 0),
                                                     stop=(kk == NH - 1))
                                noT = csb.tile([dout, 512], BF16, tag="noT")
                                nc.scalar.activation(out=noT[:, :bw],
                                                     in_=pno[:dout, :bw],
                                                     func=PRELU,
                                                     bias=W[f"b2b{li}"][:, :1],
                                                     alpha=ALPHA)
                                if li < 3:
                                    for q in range(bw // P):
                                        t_glob = tiles[(o0 // P) + q]
                                        ptr2 = ctr.tile([P, P], BF16,
                                                        space="PSUM", tag="ptr")
                                        nc.tensor.transpose(
                                            out=ptr2[:],
                                            in_=noT[:, q * P:(q + 1) * P],
                                            identity=W["ident"][:])
                                        nc.vector.tensor_copy(
                                            out=tabRM_b[:, t_glob * P:(t_glob + 1) * P],
                                            in_=ptr2[:])
                                else:
                                    phd = cmlp.tile([P, 512], F32, space="PSUM",
                                                    tag="p2", name="phd")
                                    nc.tensor.matmul(out=phd[:4, :bw],
                                                     lhsT=W["wbb"][:],
                                                     rhs=noT[:, :bw], start=True,
                                                     stop=True)
                                    ho = csb.tile([4, 512], F32, tag="ho")
                                    nc.scalar.activation(out=ho[:, :bw],
                                                         in_=phd[:4, :bw],
                                                         func=PRELU,
                                                         bias=W["bbb"][:, :1],
                                                         alpha=ALPHA)
                                    nc.sync.dma_start(
                                        out=out[:, tiles[0] * P + o0:
                                                tiles[0] * P + o0 + bw],
                                        in_=ho[:, :bw])

                    # write next layer's table to DRAM for slab gathers
                    if li < 3:
                        nc.sync.dma_start(
                            out=agins[li + 1][:, :]
                            .rearrange("(a p) d -> p a d", p=P),
                            in_=tabRM_b[:, :]
                            .rearrange("p (a d) -> p a d", d=P))

    nc.compile()
    return nc


# ---------------------------------------------------------------------------
# Entry point
# ---------------------------------------------------------------------------

_CACHE = {}


def kernel(**inputs) -> np.ndarray:
    cfg, in_maps = preprocess(inputs)
    key = hashlib.sha256(repr(sorted(cfg.items())).encode()).hexdigest()
    if key not in _CACHE:
        _CACHE[key] = build_kernel(cfg)
    nc = _CACHE[key]
    res = run_bass_kernel_spmd(nc, in_maps, list(range(NC)))
    O, OS = cfg["O"], cfg["OS"]
    full = np.zeros((4, O), np.float32)
    for c in range(NC):
        full[:, c * OS:(c + 1) * OS] = res.results[c]["out"][:, :OS]
    return np.ascontiguousarray(full.T)
